# revision 10
# baseline (speedup 1.0000x reference)
"""OTTT fused Dense+LIF spike step on 8 trn2 NeuronCores.

out = ((x @ W + b + 0.5*u0) >= 1.0).astype(f32)   x:[2048,4096] W:[4096,4096]

Sharding: data-parallel over batch (2048 -> 8 x 256 rows). W, b replicated.
Per core: x_s^T formed on-chip with PE transposes, W streamed as the moving
matmul operand in [128,512] slabs, epilogue fused into 2 DVE ops.
"""

import os

import numpy as np

B = 2048
D = 4096
NCORES = 8
BC = B // NCORES  # rows per core

# float32r: PE "replicated fp32" matmul mode, ~4x faster than plain fp32.
MM_DTYPE = os.environ.get("OTTT_MM_DTYPE", "float32")

LAST_RESULTS = None  # stash of BassKernelResults for test.py introspection
_NC_CACHE = {}


def build_nc(bc=BC, d=D, n_tile=512, mm_dtype=MM_DTYPE, reps=1):
    """Build the per-core bass program (SPMD: every core runs this).

    reps>1 repeats the whole body (for reps-differencing timing in test.py).
    """
    import concourse.bass as bass
    import concourse.mybir as mybir
    import concourse.tile as tile
    from concourse import bacc
    from concourse.alu_op_type import AluOpType
    from concourse.masks import make_identity

    f32 = mybir.dt.float32
    mmdt = getattr(mybir.dt, mm_dtype)
    P = 128
    MT = bc // P        # batch tiles per core
    KT = d // P         # contraction tiles
    NT = d // n_tile    # output-column tiles

    nc = bacc.Bacc(None, target_bir_lowering=False)
    x = nc.dram_tensor("x", [bc, d], f32, kind="ExternalInput")
    w = nc.dram_tensor("w", [d, d], f32, kind="ExternalInput")
    bvec = nc.dram_tensor("b", [d], f32, kind="ExternalInput")
    u0 = nc.dram_tensor("u0", [bc, d], f32, kind="ExternalInput")
    out = nc.dram_tensor("out", [bc, d], f32, kind="ExternalOutput")

    with tile.TileContext(nc) as tc:
        with (
            tc.tile_pool(name="const", bufs=1) as const,
            tc.tile_pool(name="xp", bufs=1) as xp,
            tc.tile_pool(name="xtp", bufs=1) as xtp,
            tc.tile_pool(name="wp", bufs=4) as wp,
            tc.tile_pool(name="up", bufs=3) as up,
            tc.tile_pool(name="sp", bufs=3) as sp,
            tc.tile_pool(name="op", bufs=3) as op,
            tc.tile_pool(name="psp", bufs=2, space="PSUM") as psp,
            tc.tile_pool(name="pst", bufs=2, space="PSUM") as pst,
        ):
            ident = const.tile([P, P], f32)
            make_identity(nc, ident[:])

            # thr[p, j] = 1 - b[j], replicated across partitions
            thr = const.tile([P, d], f32)
            b_bcast = bass.AP(bvec, 0, [[0, P], [1, d]])
            nc.gpsimd.dma_start(out=thr[:], in_=b_bcast)
            nc.vector.tensor_scalar(
                out=thr[:], in0=thr[:], scalar1=-1.0, scalar2=1.0,
                op0=AluOpType.mult, op1=AluOpType.add,
            )

            for _rep in range(reps):
                # load x, transpose to xt[k_part, tile, b_col] via PE
                x_sb = xp.tile([P, MT, d], f32)
                for m in range(MT):
                    nc.sync.dma_start(x_sb[:, m, :], x[m * P:(m + 1) * P, :])
                xt = xtp.tile([P, MT * KT, P], f32)
                for ko in range(KT):
                    for m in range(MT):
                        tp = pst.tile([P, P], f32)
                        nc.tensor.transpose(
                            tp[:], x_sb[:, m, ko * P:(ko + 1) * P], ident[:]
                        )
                        nc.vector.tensor_copy(xt[:, ko * MT + m, :], tp[:])

                for n in range(NT):
                    nsl = slice(n * n_tile, (n + 1) * n_tile)
                    ps = [
                        psp.tile([P, n_tile], f32, name=f"ps{m}")
                        for m in range(MT)
                    ]
                    for ko in range(KT):
                        wt = wp.tile([P, n_tile], f32)
                        nc.sync.dma_start(wt[:], w[ko * P:(ko + 1) * P, nsl])
                        for m in range(MT):
                            nc.tensor.matmul(
                                ps[m][:],
                                xt[:, ko * MT + m, :].bitcast(mmdt),
                                wt[:].bitcast(mmdt),
                                start=(ko == 0),
                                stop=(ko == KT - 1),
                            )
                    for m in range(MT):
                        msl = slice(m * P, (m + 1) * P)
                        ut = up.tile([P, n_tile], f32)
                        nc.sync.dma_start(ut[:], u0[msl, nsl])
                        st = sp.tile([P, n_tile], f32)
                        nc.vector.scalar_tensor_tensor(
                            out=st[:], in0=ut[:], scalar=0.5, in1=ps[m][:],
                            op0=AluOpType.mult, op1=AluOpType.add,
                        )
                        ot = op.tile([P, n_tile], f32)
                        nc.vector.tensor_tensor(
                            out=ot[:], in0=st[:], in1=thr[:, nsl],
                            op=AluOpType.is_ge,
                        )
                        nc.sync.dma_start(out[msl, nsl], ot[:])

    nc.compile()
    return nc


def make_in_maps(x, W, b, u0):
    x = np.ascontiguousarray(np.asarray(x, dtype=np.float32))
    W = np.ascontiguousarray(np.asarray(W, dtype=np.float32))
    b = np.ascontiguousarray(np.asarray(b, dtype=np.float32))
    u0 = np.ascontiguousarray(np.asarray(u0, dtype=np.float32))
    return [
        {
            "x": x[c * BC:(c + 1) * BC],
            "w": W,
            "b": b,
            "u0": u0[c * BC:(c + 1) * BC],
        }
        for c in range(NCORES)
    ]


def kernel(x, W, b, u0, a_hat0=None, **_unused):
    global LAST_RESULTS
    os.environ["BASS_NEVER_TRACE"] = "1"  # no NTFF hook in this environment
    from concourse.bass_utils import run_bass_kernel_spmd

    key = ("full", MM_DTYPE)
    if key not in _NC_CACHE:
        _NC_CACHE[key] = build_nc()
    nc = _NC_CACHE[key]

    in_maps = make_in_maps(x, W, b, u0)
    res = run_bass_kernel_spmd(nc, in_maps, list(range(NCORES)))
    LAST_RESULTS = res
    return np.concatenate([res.results[c]["out"] for c in range(NCORES)], axis=0)
